# revision 37
# baseline (speedup 1.0000x reference)
"""Causal self-attention (B=2, L=2048, D=1024, H=16, dh=64) on 8 TRN2 NeuronCores.

Sharding: core c handles batch c//4 and heads [4*(c%4), 4*(c%4)+4).
Weights are column/row-sliced per core on the host; each core computes a
partial (L, D) output through its 4 heads; the host sums the 4 partials per
batch and adds the (b_v @ W_o + b_o) bias row, which folds out of the device
kernel entirely.

Host pre-transposes x to x^T (channel-major) and downcasts x / weights to
bf16, so the device does no PE transposes and no dtype-conversion copies:
DRAM params are declared bf16 and DMA'd straight into matmul-ready tiles.
The y output is written bf16 and upcast on the host.

Device kernel per core, software-pipelined over l-blocks of 512 so the
ScalarE exp work of attention hides under the PE projection work of the next
l-block:
  A. K^T/Q^T projections in [channel-on-partition, L] layout straight from
     the persistent x^T tile; V natural [m, dh] augmented with a ones column
     (initialized once). All attention operands are bf16 (1 cycle/row on PE
     at any free size, vs fp32r's 4x penalty below 256).
  B. Attention: S^T tile [m-chunk 128, l-block 512] per head; the two heads
     of a chunk go to adjacent row-tiles (K=64 at partition 0 / 64) of the
     same PSUM pair; exp on ScalarE (scale fused, no max subtraction --
     scores are provably < ~3); diagonal-crossing chunks compute only the
     [off:] column range (the rest is never read) and get their triangle
     zeroed post-exp by a 128-wide gpsimd affine_select; O^T accumulates
     with lhsT=[V|ones] so the softmax denominator falls out as row 64 of
     the same matmul, with the same [off:] trimming.
  C. Denominators broadcast across partitions with a small selector matmul
     (own PSUM bank, so PSUM-pool reuse never head-blocks PE on the
     reciprocal); reciprocal + one merged in-place multiply normalizes U^T;
     y-projection and DMA out within the same l-block iteration.

Scheduling notes (these drove most of the speedup over a naive schedule):
  - PE dispatches strictly in order past semaphore waits: a stalled
    instruction at the queue head idles the whole engine. All tiles are
    split per-l-block and PSUM pools sized so no filler ever waits on a
    newer producer.
  - Dummy warm-up matmuls ramp the PE p-state (full clock needs 3us of
    continuous busy) while the startup DMAs land; startup DMAs are ordered
    and dc-split to feed the first projections piecewise.
  - All y-projection work is deferred (py schedule) into the last l-block,
    whose attention is otherwise exp-bound on ScalarE; the final extraction
    splits across ScalarE+DVE, the last normalize runs in halves, and the
    tail y DMAs are row-sized except the final strip.
"""

import numpy as np
import ml_dtypes

import concourse.bass as bass
import concourse.mybir as mybir
from concourse import bacc
from concourse.bass_utils import run_bass_kernel_spmd
from concourse.tile import TileContext

# Problem shape (hardcoded per contest contract).
B, L, D = 2, 2048, 1024
H, DH = 16, 64
NCORES = 8
HPC = 4  # heads per core
CSL = HPC * DH  # 256: per-core channel slice
P = 128
NDC = D // P  # 8 D-chunks
LB = 512  # l-block width
NLB = L // LB  # 4
NSTRIP = L // P  # 16
SCALE = 1.0 / float(np.sqrt(DH))

F32 = mybir.dt.float32
F32R = mybir.dt.float32r
BF16 = mybir.dt.bfloat16
EXP = mybir.ActivationFunctionType.Exp
COPY = mybir.ActivationFunctionType.Copy
ADD = mybir.AluOpType.add
MULT = mybir.AluOpType.mult


def build_nc(pull_a: int = 14, et_bufs: int = 6, pa_bufs: int = 2, pot_bufs: int = 2,
             py=(0, 0, 0, 2), lag: int = 2, warm: int = 12, py3=(1, 2),
             taper: int = 0, pool_ms: bool = False, ep: int = 2, xsplit: int = 0,
             fyx: int = 4, diag_last: bool = False, rps_pot: bool = True,
             fin_pot: bool = False, v_last: bool = True):
    nc = bacc.Bacc(None, target_bir_lowering=False, debug=False)
    xt = nc.declare_dram_parameter("xt", [D, L], BF16, isOutput=False)
    wk = nc.declare_dram_parameter("wk", [D, CSL], BF16, isOutput=False)
    wq = nc.declare_dram_parameter("wq", [D, CSL], BF16, isOutput=False)
    wv = nc.declare_dram_parameter("wv", [D, CSL], BF16, isOutput=False)
    wo = nc.declare_dram_parameter("wo", [CSL, D], BF16, isOutput=False)
    bk = nc.declare_dram_parameter("bk", [CSL], F32, isOutput=False)
    bq = nc.declare_dram_parameter("bq", [CSL], F32, isOutput=False)
    y = nc.declare_dram_parameter("y", [L, D], BF16, isOutput=True)

    with TileContext(nc) as tc:
        with (
            tc.tile_pool(name="singles", bufs=1) as singles,
            tc.tile_pool(name="work", bufs=4) as work,
            tc.tile_pool(name="exp", bufs=et_bufs) as expp,
            tc.tile_pool(name="pa", bufs=pa_bufs, space="PSUM") as pa,
            tc.tile_pool(name="psp", bufs=2, space="PSUM") as psp,
            tc.tile_pool(name="pot", bufs=pot_bufs, space="PSUM") as pot,
        ):
            # ---------- weights + x^T: DMA directly into matmul dtypes ----------
            # order: wk then the first x^T half-block so the K-projection of
            # l-block 0 starts as early as possible.
            wkr = singles.tile([P, NDC, CSL], BF16)
            wqr = singles.tile([P, NDC, CSL], BF16)
            wvr = singles.tile([P, NDC, CSL], BF16)
            wor = singles.tile([P, 2, D], BF16)
            xts = singles.tile([P, NDC, L], BF16)
            xt_r = xt.ap().rearrange("(o p) l -> p o l", p=P)
            bkq = singles.tile([P, 2, 2], F32)

            wk_r = wk.ap().rearrange("(o p) c -> p o c", p=P)
            wq_r = wq.ap().rearrange("(o p) c -> p o c", p=P)
            nc.sync.dma_start(wkr[:, 0:4, :], wk_r[:, 0:4, :])
            nc.sync.dma_start(xts[:, 0:4, 0:LB], xt_r[:, 0:4, 0:LB])
            nc.sync.dma_start(bkq[:, 0, :].unsqueeze(2), bk.ap().rearrange("(o p) -> p o", p=P).unsqueeze(2))
            nc.sync.dma_start(bkq[:, 1, :].unsqueeze(2), bq.ap().rearrange("(o p) -> p o", p=P).unsqueeze(2))
            nc.sync.dma_start(wkr[:, 4:8, :], wk_r[:, 4:8, :])
            nc.sync.dma_start(xts[:, 4:8, 0:LB], xt_r[:, 4:8, 0:LB])
            if v_last:
                nc.sync.dma_start(wqr[:, 0:4, :], wq_r[:, 0:4, :])
                nc.sync.dma_start(wvr[:], wv.ap().rearrange("(o p) c -> p o c", p=P))
            else:
                nc.sync.dma_start(wvr[:], wv.ap().rearrange("(o p) c -> p o c", p=P))
                nc.sync.dma_start(wqr[:, 0:4, :], wq_r[:, 0:4, :])
            nc.sync.dma_start(wqr[:, 4:8, :], wq_r[:, 4:8, :])
            for lb in range(1, NLB):
                nc.sync.dma_start(
                    xts[:, :, lb * LB : (lb + 1) * LB], xt_r[:, :, lb * LB : (lb + 1) * LB]
                )
            nc.sync.dma_start(wor[:], wo.ap().rearrange("(o p) c -> p o c", p=P))

            # ---------- constants ----------
            # denominator staging rows 64 (even head) / 96 (odd head).
            # One tile per (lb+cc)%4 slot so hazards stay per-slot.
            dsbs = [singles.tile([P, LB], BF16, name=f"dsb{i}") for i in range(4)]
            (nc.gpsimd if pool_ms else nc.vector).memset(dsbs[0][:], 0.0)

            # ramp the PE p-state with dependency-free dummy matmuls while
            # the startup DMAs land (their results are never read; the source
            # is deliberately uninitialized - only the ramp matters)
            if warm:
                wsrc = singles.tile([P, LB], BF16, name="warmsrc")
                nc.gpsimd.memset(wsrc[:], 0.0)
                wscr = pa.tile([P, LB], F32, tag="pa", name="warmscr")
                for i in range(warm):
                    nc.tensor.matmul(
                        wscr[:], wsrc[:, 0:P], wsrc[:], start=(i == 0),
                        stop=True, skip_group_check=True,
                    )

            for d in dsbs[1:]:
                (nc.gpsimd if pool_ms else nc.vector).memset(d[:], 0.0)
            ones1 = singles.tile([P, 1], F32)
            nc.vector.memset(ones1[:], 1.0)

            # denominator-broadcast selector (fp32r; 0/1 are exact)
            e32 = singles.tile([P, P], F32)
            nc.vector.memset(e32[:], 0.0)
            nc.vector.memset(e32[64:65, 0:64], 1.0)
            nc.vector.memset(e32[96:97, 64:128], 1.0)
            e32r = singles.tile([P, P], BF16)
            nc.vector.tensor_copy(e32r[:], e32[:])

            # ---------- persistent activations ----------
            # Per-l-block tiles: a single big tile would create false
            # cross-block hazards (the tracker is coarse), head-blocking PE.
            kts = [singles.tile([P, 2, LB], BF16, name=f"kt{i}") for i in range(NLB)]
            qts = [singles.tile([P, 2, LB], BF16, name=f"qt{i}") for i in range(NLB)]
            vos = [
                singles.tile([P, 4, HPC, DH + 1], BF16, name=f"vo{i}")
                for i in range(NLB)
            ]
            ucats = [
                singles.tile([P, 2, LB], BF16, name=f"ucat{i}") for i in range(NLB)
            ]
            for v in vos:
                nc.vector.tensor_copy(
                    v[:, :, :, DH : DH + 1], ones1[:].to_broadcast((P, 4, HPC, 1))
                )

            def gen_A(lb, js=(0, 1), with_v=True):
                """Phase A (K/Q/V projections) of l-block lb, yielded as
                small units so it can be interleaved under attention."""
                l0 = lb * LB
                lsl = slice(l0, l0 + LB)

                def proj(which, wr, dst):
                    for j in js:
                        pj = pa.tile([P, LB], F32, tag="pa", name=f"pj{lb}_{which}_{j}")
                        for dc in range(NDC):
                            nc.tensor.matmul(
                                pj[:],
                                wr[:, dc, j * P : (j + 1) * P],
                                xts[:, dc, lsl],
                                start=(dc == 0),
                                stop=(dc == NDC - 1),
                            )
                            yield
                        nc.vector.tensor_scalar(
                            out=dst[:, j, :],
                            in0=pj[:],
                            scalar1=bkq[:, which, j : j + 1],
                            scalar2=None,
                            op0=ADD,
                        )

                yield from proj(0, wkr, kts[lb])
                if v_last:
                    yield from proj(1, wqr, qts[lb])
                for ml in range(4 if with_v else 0):
                    mc = lb * 4 + ml
                    pv = pa.tile([P, CSL], F32, tag="pa", name=f"pv{lb}_{ml}")
                    for dc in range(NDC):
                        nc.tensor.matmul(
                            pv[:],
                            xts[:, dc, l0 + ml * P : l0 + (ml + 1) * P],
                            wvr[:, dc, :],
                            start=(dc == 0),
                            stop=(dc == NDC - 1),
                        )
                        yield
                    nc.vector.tensor_copy(
                        vos[lb][:, ml, :, 0:DH],
                        pv[:].rearrange("p (h d) -> p h d", h=HPC),
                    )
                if not v_last:
                    yield from proj(1, wqr, qts[lb])

            def gen_Y(lb):
                """y-projection of l-block lb (normalize(lb) must be emitted)."""
                l0 = lb * LB
                last = lb == NLB - 1
                for sl in range(4):
                    r0 = l0 + sl * P
                    ys = work.tile([P, D], BF16, tag="ys", name=f"ys{lb}_{sl}")
                    for jt in range(2):
                        if last and jt == 1:
                            ypt = psp.tile([P, 2, LB], F32, tag="sps")
                            yp = ypt[:, 0, :]
                        elif last and fin_pot:
                            yp = pot.tile([P, LB], F32, tag="ot", name=f"yp{lb}_{sl}_{jt}")[:]
                        else:
                            yp = pa.tile([P, LB], F32, tag="pa", name=f"yp{lb}_{sl}_{jt}")[:]
                        for cc in range(2):
                            nc.tensor.matmul(
                                yp,
                                ucats[lb][:, cc, sl * P : (sl + 1) * P],
                                wor[:, cc, jt * LB : (jt + 1) * LB],
                                start=(cc == 0),
                                stop=(cc == 1),
                            )
                            yield
                        dst = ys[:, jt * LB : (jt + 1) * LB]
                        if last and jt == 1:
                            nc.scalar.activation(out=dst, in_=yp, func=COPY)
                        else:
                            nc.vector.tensor_copy(dst, yp)
                        yield
                        if last and sl == 3:
                            nc.sync.dma_start(
                                y.ap()[r0 : r0 + P, jt * LB : (jt + 1) * LB],
                                ys[:, jt * LB : (jt + 1) * LB],
                            )
                            yield
                    if not (last and sl == 3):
                        nc.sync.dma_start(y.ap()[r0 : r0 + P, :], ys[:])
                        yield

            def pull(gens, k):
                n = 0
                while n < k and gens:
                    try:
                        next(gens[0])
                        n += 1
                    except StopIteration:
                        gens.pop(0)

            filler_a = [gen_A(0, js=(0,), with_v=True)]
            filler_y = []
            pull(filler_a, 10**9)  # j0+V of lb=0 runs un-interleaved
            filler_a = [gen_A(0, js=(1,), with_v=False)]
            PY = py  # per-lb y-filler pull rate: save y work for lb3

            for lb in range(NLB):
                l0 = lb * LB
                lsl = slice(l0, l0 + LB)
                if lb + 1 < NLB:
                    filler_a.append(gen_A(lb + 1))

                # ===== attention for this l-block (diagonal chunks first) =====
                for cc in range(2):
                    final_attn = lb == NLB - 1 and cc == 1
                    if lb == NLB - 1:
                        pull_y = py3[cc]
                    else:
                        pull_y = PY[lb]
                    ots = [
                        pot.tile([P, LB], F32, tag="ot", name=f"ot_{cc}_{lb}_{par}")
                        for par in range(2)
                    ]
                    nmc = 4 * (lb + 1)
                    if diag_last and lb > 0:
                        mc_order = list(range(4 * lb)) + list(range(4 * lb, nmc))
                    else:
                        mc_order = list(range(4 * lb, nmc)) + list(range(4 * lb))
                    pend = []  # (mc, et) waiting for the lag-1 O^T

                    def emit_ot(last):
                        omc, oet = pend.pop(0)
                        first = omc == mc_order[0]
                        off = omc * P - l0 if omc >= 4 * lb else 0
                        for par in range(2):
                            hl = 2 * cc + par
                            nc.tensor.matmul(
                                ots[par][0 : DH + 1, off:LB],
                                vos[omc // 4][:, omc % 4, hl, :],
                                oet[:, par, off:LB],
                                start=first,
                                stop=last,
                                skip_group_check=True,
                            )

                    for step, mc in enumerate(mc_order):
                        sp = psp.tile([P, 2, LB], F32, tag="sps")
                        et = expp.tile([P, 2, LB], BF16, tag="et")
                        if mc >= 4 * lb:  # diagonal-crossing m-chunk
                            # columns < off are never read: compute [off:] only
                            off = mc * P - l0
                            for par in range(2):
                                hb = 64 * par
                                nc.tensor.matmul(
                                    sp[:, par, off:LB],
                                    kts[mc // 4][hb : hb + 64, cc, (mc % 4) * P : (mc % 4 + 1) * P],
                                    qts[lb][hb : hb + 64, cc, off:LB],
                                    start=True,
                                    stop=True,
                                )
                            nc.scalar.activation(
                                out=et[:, :, off:LB],
                                in_=sp[:, :, off:LB],
                                func=EXP,
                                scale=SCALE,
                            )
                            # zero the above-diagonal triangle of the 128-wide
                            # diagonal window
                            nc.gpsimd.affine_select(
                                out=et[:, :, off : off + P],
                                in_=et[:, :, off : off + P],
                                compare_op=mybir.AluOpType.is_ge,
                                fill=0.0,
                                base=0,
                                pattern=[[0, 2], [1, P]],
                                channel_multiplier=-1,
                            )
                        else:
                            for par in range(2):
                                hb = 64 * par
                                nc.tensor.matmul(
                                    sp[:, par, :],
                                    kts[mc // 4][hb : hb + 64, cc, (mc % 4) * P : (mc % 4 + 1) * P],
                                    qts[lb][hb : hb + 64, cc, :],
                                    start=True,
                                    stop=True,
                                )
                            nc.scalar.activation(
                                out=et[:], in_=sp[:], func=EXP, scale=SCALE
                            )
                        pend.append((mc, et))
                        if step > lag:
                            emit_ot(last=False)  # lagged: its exp is done
                        # taper y pulls near the end of the final cc so the
                        # DVE queue is drained when the extraction copies come
                        if not (final_attn and taper and step >= len(mc_order) - taper):
                            pull(filler_y, pull_y)
                        pull(filler_a, pull_a)
                    while len(pend) > 1:
                        emit_ot(last=False)
                        pull(filler_y, 1)
                        pull(filler_a, 1)
                    emit_ot(last=True)
                    # denominators + unnormalized U^T into place, interleaved
                    # with filler so at most 2 DVE ops wait in the queue
                    final_cc = lb == NLB - 1 and cc == 1
                    dsb = dsbs[(2 * lb + cc) % 4]
                    if final_cc or xsplit:
                        # denominators first (they gate rps->recip), split
                        # across DVE/ACT; U^T extraction on the other engine
                        nc.vector.tensor_copy(dsb[64:65, :], ots[0][DH : DH + 1, :])
                        nc.scalar.activation(
                            out=dsb[96:97, :], in_=ots[1][DH : DH + 1, :], func=COPY
                        )
                        nc.scalar.activation(
                            out=ucats[lb][0:DH, cc, :], in_=ots[0][0:DH, :], func=COPY
                        )
                        nc.vector.tensor_copy(
                            ucats[lb][64 : 64 + DH, cc, :], ots[1][0:DH, :]
                        )
                        pull(filler_y, fyx)
                        pull(filler_a, 4)
                    else:
                        for par in range(2):
                            nc.vector.tensor_copy(
                                dsb[64 + 32 * par : 65 + 32 * par, :],
                                ots[par][DH : DH + 1, :],
                            )
                            nc.vector.tensor_copy(
                                ucats[lb][64 * par : 64 * par + 64, cc, :],
                                ots[par][0:DH, :],
                            )
                            pull(filler_y, 1)
                            pull(filler_a, ep)
                    # normalize this chunk (overlaps the other chunk's attention)
                    rpool = pot if rps_pot else pa
                    rps = rpool.tile([P, LB], F32, tag="ot" if rps_pot else "pa", name=f"rps{lb}_{cc}")
                    nc.tensor.matmul(
                        rps[:], e32r[64:128, :], dsb[64:128, :],
                        start=True, stop=True,
                    )
                    pull(filler_y, 1)
                    pull(filler_a, ep)
                    rr = work.tile([P, LB], BF16, tag="rr", name=f"rr{lb}_{cc}")
                    halves = (slice(0, LB // 2), slice(LB // 2, LB)) if final_cc else (slice(0, LB),)
                    with nc.allow_low_precision(reason="f32r recip feeds f32r normalize"):
                        for h in halves:
                            nc.vector.reciprocal(rr[:, h], rps[:, h])
                    pull(filler_a, 2)
                    for h in halves:
                        nc.vector.tensor_tensor(
                            out=ucats[lb][:, cc, h],
                            in0=ucats[lb][:, cc, h],
                            in1=rr[:, h],
                            op=MULT,
                        )
                # next l-block's projections must not gate its attention
                pull(filler_a, 10**9)
                filler_y.append(gen_Y(lb))
            pull(filler_y, 10**9)

    nc.finalize()
    return nc


_NC = None


def _get_nc():
    global _NC
    if _NC is None:
        _NC = build_nc()
    return _NC


def _perm_kq(head_base: int) -> np.ndarray:
    """Channel permutation mapping device layout (chunk j, partition p) ->
    global channel (head_base + 2j + (p>=64))*64 + p%64."""
    idx = np.empty(CSL, dtype=np.int64)
    for j in range(2):
        for p in range(P):
            idx[j * P + p] = (head_base + 2 * j + (1 if p >= 64 else 0)) * DH + (p % 64)
    return idx


def make_in_maps(x, W_kq, b_kq, W_v, b_v, W_o, b_o):
    bf = ml_dtypes.bfloat16
    in_maps = []
    xts = [np.ascontiguousarray(x[b].T.astype(bf)) for b in range(B)]
    for c in range(NCORES):
        b = c // 4
        head_base = 4 * (c % 4)
        perm = _perm_kq(head_base)
        in_maps.append(
            {
                "xt": xts[b],
                "wk": np.ascontiguousarray(W_kq[:, perm].astype(bf)),
                "wq": np.ascontiguousarray(W_kq[:, D + perm].astype(bf)),
                "wv": np.ascontiguousarray(
                    W_v[:, head_base * DH : head_base * DH + CSL].astype(bf)
                ),
                "wo": np.ascontiguousarray(W_o[perm, :].astype(bf)),
                "bk": np.ascontiguousarray(b_kq[perm]),
                "bq": np.ascontiguousarray(b_kq[D + perm]),
            }
        )
    return in_maps


def assemble(results, b_v, W_o, b_o):
    bias_row = (b_v.astype(np.float64) @ W_o.astype(np.float64) + b_o).astype(
        np.float32
    )
    out = np.zeros((B, L, D), dtype=np.float32)
    for c in range(NCORES):
        out[c // 4] += np.asarray(results[c]["y"]).astype(np.float32)
    out += bias_row[None, None, :]
    return out


def kernel(x, W_kq, b_kq, W_v, b_v, W_o, b_o):
    x = np.asarray(x, dtype=np.float32)
    W_kq = np.asarray(W_kq, dtype=np.float32)
    b_kq = np.asarray(b_kq, dtype=np.float32)
    W_v = np.asarray(W_v, dtype=np.float32)
    b_v = np.asarray(b_v, dtype=np.float32)
    W_o = np.asarray(W_o, dtype=np.float32)
    b_o = np.asarray(b_o, dtype=np.float32)

    nc = _get_nc()
    in_maps = make_in_maps(x, W_kq, b_kq, W_v, b_v, W_o, b_o)
    res = run_bass_kernel_spmd(nc, in_maps, core_ids=list(range(NCORES)))
    return assemble(res.results, b_v, W_o, b_o)


# revision 38
# speedup vs baseline: 1.0004x; 1.0004x over previous
"""Causal self-attention (B=2, L=2048, D=1024, H=16, dh=64) on 8 TRN2 NeuronCores.

Sharding: core c handles batch c//4 and heads [4*(c%4), 4*(c%4)+4).
Weights are column/row-sliced per core on the host; each core computes a
partial (L, D) output through its 4 heads; the host sums the 4 partials per
batch and adds the (b_v @ W_o + b_o) bias row, which folds out of the device
kernel entirely.

Host pre-transposes x to x^T (channel-major) and downcasts x / weights to
bf16, so the device does no PE transposes and no dtype-conversion copies:
DRAM params are declared bf16 and DMA'd straight into matmul-ready tiles.
The y output is written bf16 and upcast on the host.

Device kernel per core, software-pipelined over l-blocks of 512 so the
ScalarE exp work of attention hides under the PE projection work of the next
l-block:
  A. K^T/Q^T projections in [channel-on-partition, L] layout straight from
     the persistent x^T tile; V natural [m, dh] augmented with a ones column
     (initialized once). All attention operands are bf16 (1 cycle/row on PE
     at any free size, vs fp32r's 4x penalty below 256).
  B. Attention: S^T tile [m-chunk 128, l-block 512] per head; the two heads
     of a chunk go to adjacent row-tiles (K=64 at partition 0 / 64) of the
     same PSUM pair; exp on ScalarE (scale fused, no max subtraction --
     scores are provably < ~3); diagonal-crossing chunks compute only the
     [off:] column range (the rest is never read) and get their triangle
     zeroed post-exp by a 128-wide gpsimd affine_select; O^T accumulates
     with lhsT=[V|ones] so the softmax denominator falls out as row 64 of
     the same matmul, with the same [off:] trimming.
  C. Denominators broadcast across partitions with a small selector matmul
     (own PSUM bank, so PSUM-pool reuse never head-blocks PE on the
     reciprocal); reciprocal + one merged in-place multiply normalizes U^T;
     y-projection and DMA out within the same l-block iteration.

Scheduling notes (these drove most of the speedup over a naive schedule):
  - PE dispatches strictly in order past semaphore waits: a stalled
    instruction at the queue head idles the whole engine. All tiles are
    split per-l-block and PSUM pools sized so no filler ever waits on a
    newer producer.
  - Dummy warm-up matmuls ramp the PE p-state (full clock needs 3us of
    continuous busy) while the startup DMAs land; startup DMAs are ordered
    and dc-split to feed the first projections piecewise.
  - All y-projection work is deferred (py schedule) into the last l-block,
    whose attention is otherwise exp-bound on ScalarE; the final extraction
    splits across ScalarE+DVE, the last normalize runs in halves, and the
    tail y DMAs are row-sized except the final strip.
"""

import numpy as np
import ml_dtypes

import concourse.bass as bass
import concourse.mybir as mybir
from concourse import bacc
from concourse.bass_utils import run_bass_kernel_spmd
from concourse.tile import TileContext

# Problem shape (hardcoded per contest contract).
B, L, D = 2, 2048, 1024
H, DH = 16, 64
NCORES = 8
HPC = 4  # heads per core
CSL = HPC * DH  # 256: per-core channel slice
P = 128
NDC = D // P  # 8 D-chunks
LB = 512  # l-block width
NLB = L // LB  # 4
NSTRIP = L // P  # 16
SCALE = 1.0 / float(np.sqrt(DH))

F32 = mybir.dt.float32
F32R = mybir.dt.float32r
BF16 = mybir.dt.bfloat16
EXP = mybir.ActivationFunctionType.Exp
COPY = mybir.ActivationFunctionType.Copy
ADD = mybir.AluOpType.add
MULT = mybir.AluOpType.mult


def build_nc(pull_a: int = 14, et_bufs: int = 6, pa_bufs: int = 2, pot_bufs: int = 2,
             py=(0, 0, 0, 2), lag: int = 2, warm: int = 12, py3=(1, 2),
             taper: int = 0, pool_ms: bool = False, ep: int = 2, xsplit: int = 0,
             fyx: int = 4, diag_last: bool = False, rps_pot: bool = True,
             fin_pot: bool = False, v_last: bool = True):
    nc = bacc.Bacc(None, target_bir_lowering=False, debug=False)
    xt = nc.declare_dram_parameter("xt", [D, L], BF16, isOutput=False)
    wk = nc.declare_dram_parameter("wk", [D, CSL], BF16, isOutput=False)
    wq = nc.declare_dram_parameter("wq", [D, CSL], BF16, isOutput=False)
    wv = nc.declare_dram_parameter("wv", [D, CSL], BF16, isOutput=False)
    wo = nc.declare_dram_parameter("wo", [CSL, D], BF16, isOutput=False)
    bk = nc.declare_dram_parameter("bk", [CSL], F32, isOutput=False)
    bq = nc.declare_dram_parameter("bq", [CSL], F32, isOutput=False)
    y = nc.declare_dram_parameter("y", [L, D], BF16, isOutput=True)

    with TileContext(nc) as tc:
        with (
            tc.tile_pool(name="singles", bufs=1) as singles,
            tc.tile_pool(name="work", bufs=4) as work,
            tc.tile_pool(name="exp", bufs=et_bufs) as expp,
            tc.tile_pool(name="pa", bufs=pa_bufs, space="PSUM") as pa,
            tc.tile_pool(name="psp", bufs=2, space="PSUM") as psp,
            tc.tile_pool(name="pot", bufs=pot_bufs, space="PSUM") as pot,
        ):
            # ---------- weights + x^T: DMA directly into matmul dtypes ----------
            # order: wk then the first x^T half-block so the K-projection of
            # l-block 0 starts as early as possible.
            wkr = singles.tile([P, NDC, CSL], BF16)
            wqr = singles.tile([P, NDC, CSL], BF16)
            wvr = singles.tile([P, NDC, CSL], BF16)
            wor = singles.tile([P, 2, D], BF16)
            xts = singles.tile([P, NDC, L], BF16)
            xt_r = xt.ap().rearrange("(o p) l -> p o l", p=P)
            bkq = singles.tile([P, 2, 2], F32)

            wk_r = wk.ap().rearrange("(o p) c -> p o c", p=P)
            wq_r = wq.ap().rearrange("(o p) c -> p o c", p=P)
            nc.sync.dma_start(wkr[:, 0:4, :], wk_r[:, 0:4, :])
            nc.sync.dma_start(xts[:, 0:4, 0:LB], xt_r[:, 0:4, 0:LB])
            nc.sync.dma_start(bkq[:, 0, :].unsqueeze(2), bk.ap().rearrange("(o p) -> p o", p=P).unsqueeze(2))
            nc.sync.dma_start(bkq[:, 1, :].unsqueeze(2), bq.ap().rearrange("(o p) -> p o", p=P).unsqueeze(2))
            nc.sync.dma_start(wkr[:, 4:8, :], wk_r[:, 4:8, :])
            nc.sync.dma_start(xts[:, 4:8, 0:LB], xt_r[:, 4:8, 0:LB])
            if v_last:
                nc.sync.dma_start(wqr[:, 0:4, :], wq_r[:, 0:4, :])
                nc.sync.dma_start(wvr[:], wv.ap().rearrange("(o p) c -> p o c", p=P))
            else:
                nc.sync.dma_start(wvr[:], wv.ap().rearrange("(o p) c -> p o c", p=P))
                nc.sync.dma_start(wqr[:, 0:4, :], wq_r[:, 0:4, :])
            nc.sync.dma_start(wqr[:, 4:8, :], wq_r[:, 4:8, :])
            nc.sync.dma_start(xts[:, 0:4, LB : 2 * LB], xt_r[:, 0:4, LB : 2 * LB])
            nc.sync.dma_start(xts[:, 4:8, LB : 2 * LB], xt_r[:, 4:8, LB : 2 * LB])
            for lb in range(2, NLB):
                nc.sync.dma_start(
                    xts[:, :, lb * LB : (lb + 1) * LB], xt_r[:, :, lb * LB : (lb + 1) * LB]
                )
            nc.sync.dma_start(wor[:], wo.ap().rearrange("(o p) c -> p o c", p=P))

            # ---------- constants ----------
            # denominator staging rows 64 (even head) / 96 (odd head).
            # One tile per (lb+cc)%4 slot so hazards stay per-slot.
            dsbs = [singles.tile([P, LB], BF16, name=f"dsb{i}") for i in range(4)]
            (nc.gpsimd if pool_ms else nc.vector).memset(dsbs[0][:], 0.0)

            # ramp the PE p-state with dependency-free dummy matmuls while
            # the startup DMAs land (their results are never read; the source
            # is deliberately uninitialized - only the ramp matters)
            if warm:
                wsrc = singles.tile([P, LB], BF16, name="warmsrc")
                nc.gpsimd.memset(wsrc[:], 0.0)
                wscr = pa.tile([P, LB], F32, tag="pa", name="warmscr")
                for i in range(warm):
                    nc.tensor.matmul(
                        wscr[:], wsrc[:, 0:P], wsrc[:], start=(i == 0),
                        stop=True, skip_group_check=True,
                    )

            for d in dsbs[1:]:
                (nc.gpsimd if pool_ms else nc.vector).memset(d[:], 0.0)
            ones1 = singles.tile([P, 1], F32)
            nc.vector.memset(ones1[:], 1.0)

            # denominator-broadcast selector (fp32r; 0/1 are exact)
            e32 = singles.tile([P, P], F32)
            nc.vector.memset(e32[:], 0.0)
            nc.vector.memset(e32[64:65, 0:64], 1.0)
            nc.vector.memset(e32[96:97, 64:128], 1.0)
            e32r = singles.tile([P, P], BF16)
            nc.vector.tensor_copy(e32r[:], e32[:])

            # ---------- persistent activations ----------
            # Per-l-block tiles: a single big tile would create false
            # cross-block hazards (the tracker is coarse), head-blocking PE.
            kts = [singles.tile([P, 2, LB], BF16, name=f"kt{i}") for i in range(NLB)]
            qts = [singles.tile([P, 2, LB], BF16, name=f"qt{i}") for i in range(NLB)]
            vos = [
                singles.tile([P, 4, HPC, DH + 1], BF16, name=f"vo{i}")
                for i in range(NLB)
            ]
            ucats = [
                singles.tile([P, 2, LB], BF16, name=f"ucat{i}") for i in range(NLB)
            ]
            for v in vos:
                nc.vector.tensor_copy(
                    v[:, :, :, DH : DH + 1], ones1[:].to_broadcast((P, 4, HPC, 1))
                )

            def gen_A(lb, js=(0, 1), with_v=True):
                """Phase A (K/Q/V projections) of l-block lb, yielded as
                small units so it can be interleaved under attention."""
                l0 = lb * LB
                lsl = slice(l0, l0 + LB)

                def proj(which, wr, dst):
                    for j in js:
                        pj = pa.tile([P, LB], F32, tag="pa", name=f"pj{lb}_{which}_{j}")
                        for dc in range(NDC):
                            nc.tensor.matmul(
                                pj[:],
                                wr[:, dc, j * P : (j + 1) * P],
                                xts[:, dc, lsl],
                                start=(dc == 0),
                                stop=(dc == NDC - 1),
                            )
                            yield
                        nc.vector.tensor_scalar(
                            out=dst[:, j, :],
                            in0=pj[:],
                            scalar1=bkq[:, which, j : j + 1],
                            scalar2=None,
                            op0=ADD,
                        )

                yield from proj(0, wkr, kts[lb])
                if v_last:
                    yield from proj(1, wqr, qts[lb])
                for ml in range(4 if with_v else 0):
                    mc = lb * 4 + ml
                    pv = pa.tile([P, CSL], F32, tag="pa", name=f"pv{lb}_{ml}")
                    for dc in range(NDC):
                        nc.tensor.matmul(
                            pv[:],
                            xts[:, dc, l0 + ml * P : l0 + (ml + 1) * P],
                            wvr[:, dc, :],
                            start=(dc == 0),
                            stop=(dc == NDC - 1),
                        )
                        yield
                    nc.vector.tensor_copy(
                        vos[lb][:, ml, :, 0:DH],
                        pv[:].rearrange("p (h d) -> p h d", h=HPC),
                    )
                if not v_last:
                    yield from proj(1, wqr, qts[lb])

            def gen_Y(lb):
                """y-projection of l-block lb (normalize(lb) must be emitted)."""
                l0 = lb * LB
                last = lb == NLB - 1
                for sl in range(4):
                    r0 = l0 + sl * P
                    ys = work.tile([P, D], BF16, tag="ys", name=f"ys{lb}_{sl}")
                    for jt in range(2):
                        if last and jt == 1:
                            ypt = psp.tile([P, 2, LB], F32, tag="sps")
                            yp = ypt[:, 0, :]
                        elif last and fin_pot:
                            yp = pot.tile([P, LB], F32, tag="ot", name=f"yp{lb}_{sl}_{jt}")[:]
                        else:
                            yp = pa.tile([P, LB], F32, tag="pa", name=f"yp{lb}_{sl}_{jt}")[:]
                        for cc in range(2):
                            nc.tensor.matmul(
                                yp,
                                ucats[lb][:, cc, sl * P : (sl + 1) * P],
                                wor[:, cc, jt * LB : (jt + 1) * LB],
                                start=(cc == 0),
                                stop=(cc == 1),
                            )
                            yield
                        dst = ys[:, jt * LB : (jt + 1) * LB]
                        if last and jt == 1:
                            nc.scalar.activation(out=dst, in_=yp, func=COPY)
                        else:
                            nc.vector.tensor_copy(dst, yp)
                        yield
                        if last and sl == 3:
                            nc.sync.dma_start(
                                y.ap()[r0 : r0 + P, jt * LB : (jt + 1) * LB],
                                ys[:, jt * LB : (jt + 1) * LB],
                            )
                            yield
                    if not (last and sl == 3):
                        nc.sync.dma_start(y.ap()[r0 : r0 + P, :], ys[:])
                        yield

            def pull(gens, k):
                n = 0
                while n < k and gens:
                    try:
                        next(gens[0])
                        n += 1
                    except StopIteration:
                        gens.pop(0)

            filler_a = [gen_A(0, js=(0,), with_v=True)]
            filler_y = []
            pull(filler_a, 10**9)  # j0+V of lb=0 runs un-interleaved
            filler_a = [gen_A(0, js=(1,), with_v=False)]
            PY = py  # per-lb y-filler pull rate: save y work for lb3

            for lb in range(NLB):
                l0 = lb * LB
                lsl = slice(l0, l0 + LB)
                if lb + 1 < NLB:
                    filler_a.append(gen_A(lb + 1))

                # ===== attention for this l-block (diagonal chunks first) =====
                for cc in range(2):
                    final_attn = lb == NLB - 1 and cc == 1
                    if lb == NLB - 1:
                        pull_y = py3[cc]
                    else:
                        pull_y = PY[lb]
                    ots = [
                        pot.tile([P, LB], F32, tag="ot", name=f"ot_{cc}_{lb}_{par}")
                        for par in range(2)
                    ]
                    nmc = 4 * (lb + 1)
                    if diag_last and lb > 0:
                        mc_order = list(range(4 * lb)) + list(range(4 * lb, nmc))
                    else:
                        mc_order = list(range(4 * lb, nmc)) + list(range(4 * lb))
                    pend = []  # (mc, et) waiting for the lag-1 O^T

                    def emit_ot(last):
                        omc, oet = pend.pop(0)
                        first = omc == mc_order[0]
                        off = omc * P - l0 if omc >= 4 * lb else 0
                        for par in range(2):
                            hl = 2 * cc + par
                            nc.tensor.matmul(
                                ots[par][0 : DH + 1, off:LB],
                                vos[omc // 4][:, omc % 4, hl, :],
                                oet[:, par, off:LB],
                                start=first,
                                stop=last,
                                skip_group_check=True,
                            )

                    for step, mc in enumerate(mc_order):
                        sp = psp.tile([P, 2, LB], F32, tag="sps")
                        et = expp.tile([P, 2, LB], BF16, tag="et")
                        if mc >= 4 * lb:  # diagonal-crossing m-chunk
                            # columns < off are never read: compute [off:] only
                            off = mc * P - l0
                            for par in range(2):
                                hb = 64 * par
                                nc.tensor.matmul(
                                    sp[:, par, off:LB],
                                    kts[mc // 4][hb : hb + 64, cc, (mc % 4) * P : (mc % 4 + 1) * P],
                                    qts[lb][hb : hb + 64, cc, off:LB],
                                    start=True,
                                    stop=True,
                                )
                            nc.scalar.activation(
                                out=et[:, :, off:LB],
                                in_=sp[:, :, off:LB],
                                func=EXP,
                                scale=SCALE,
                            )
                            # zero the above-diagonal triangle of the 128-wide
                            # diagonal window
                            nc.gpsimd.affine_select(
                                out=et[:, :, off : off + P],
                                in_=et[:, :, off : off + P],
                                compare_op=mybir.AluOpType.is_ge,
                                fill=0.0,
                                base=0,
                                pattern=[[0, 2], [1, P]],
                                channel_multiplier=-1,
                            )
                        else:
                            for par in range(2):
                                hb = 64 * par
                                nc.tensor.matmul(
                                    sp[:, par, :],
                                    kts[mc // 4][hb : hb + 64, cc, (mc % 4) * P : (mc % 4 + 1) * P],
                                    qts[lb][hb : hb + 64, cc, :],
                                    start=True,
                                    stop=True,
                                )
                            nc.scalar.activation(
                                out=et[:], in_=sp[:], func=EXP, scale=SCALE
                            )
                        pend.append((mc, et))
                        if step > lag:
                            emit_ot(last=False)  # lagged: its exp is done
                        # taper y pulls near the end of the final cc so the
                        # DVE queue is drained when the extraction copies come
                        if not (final_attn and taper and step >= len(mc_order) - taper):
                            pull(filler_y, pull_y)
                        pull(filler_a, pull_a)
                    while len(pend) > 1:
                        emit_ot(last=False)
                        pull(filler_y, 1)
                        pull(filler_a, 1)
                    emit_ot(last=True)
                    # denominators + unnormalized U^T into place, interleaved
                    # with filler so at most 2 DVE ops wait in the queue
                    final_cc = lb == NLB - 1 and cc == 1
                    dsb = dsbs[(2 * lb + cc) % 4]
                    if final_cc or xsplit:
                        # denominators first (they gate rps->recip), split
                        # across DVE/ACT; U^T extraction on the other engine
                        nc.vector.tensor_copy(dsb[64:65, :], ots[0][DH : DH + 1, :])
                        nc.scalar.activation(
                            out=dsb[96:97, :], in_=ots[1][DH : DH + 1, :], func=COPY
                        )
                        nc.scalar.activation(
                            out=ucats[lb][0:DH, cc, :], in_=ots[0][0:DH, :], func=COPY
                        )
                        nc.vector.tensor_copy(
                            ucats[lb][64 : 64 + DH, cc, :], ots[1][0:DH, :]
                        )
                        pull(filler_y, fyx)
                        pull(filler_a, 4)
                    else:
                        for par in range(2):
                            nc.vector.tensor_copy(
                                dsb[64 + 32 * par : 65 + 32 * par, :],
                                ots[par][DH : DH + 1, :],
                            )
                            nc.vector.tensor_copy(
                                ucats[lb][64 * par : 64 * par + 64, cc, :],
                                ots[par][0:DH, :],
                            )
                            pull(filler_y, 1)
                            pull(filler_a, ep)
                    # normalize this chunk (overlaps the other chunk's attention)
                    rpool = pot if rps_pot else pa
                    rps = rpool.tile([P, LB], F32, tag="ot" if rps_pot else "pa", name=f"rps{lb}_{cc}")
                    nc.tensor.matmul(
                        rps[:], e32r[64:128, :], dsb[64:128, :],
                        start=True, stop=True,
                    )
                    pull(filler_y, 1)
                    pull(filler_a, ep)
                    rr = work.tile([P, LB], BF16, tag="rr", name=f"rr{lb}_{cc}")
                    halves = (slice(0, LB // 2), slice(LB // 2, LB)) if final_cc else (slice(0, LB),)
                    with nc.allow_low_precision(reason="f32r recip feeds f32r normalize"):
                        for h in halves:
                            nc.vector.reciprocal(rr[:, h], rps[:, h])
                    pull(filler_a, 2)
                    for h in halves:
                        nc.vector.tensor_tensor(
                            out=ucats[lb][:, cc, h],
                            in0=ucats[lb][:, cc, h],
                            in1=rr[:, h],
                            op=MULT,
                        )
                # next l-block's projections must not gate its attention
                pull(filler_a, 10**9)
                filler_y.append(gen_Y(lb))
            pull(filler_y, 10**9)

    nc.finalize()
    return nc


_NC = None


def _get_nc():
    global _NC
    if _NC is None:
        _NC = build_nc()
    return _NC


def _perm_kq(head_base: int) -> np.ndarray:
    """Channel permutation mapping device layout (chunk j, partition p) ->
    global channel (head_base + 2j + (p>=64))*64 + p%64."""
    idx = np.empty(CSL, dtype=np.int64)
    for j in range(2):
        for p in range(P):
            idx[j * P + p] = (head_base + 2 * j + (1 if p >= 64 else 0)) * DH + (p % 64)
    return idx


def make_in_maps(x, W_kq, b_kq, W_v, b_v, W_o, b_o):
    bf = ml_dtypes.bfloat16
    in_maps = []
    xts = [np.ascontiguousarray(x[b].T.astype(bf)) for b in range(B)]
    for c in range(NCORES):
        b = c // 4
        head_base = 4 * (c % 4)
        perm = _perm_kq(head_base)
        in_maps.append(
            {
                "xt": xts[b],
                "wk": np.ascontiguousarray(W_kq[:, perm].astype(bf)),
                "wq": np.ascontiguousarray(W_kq[:, D + perm].astype(bf)),
                "wv": np.ascontiguousarray(
                    W_v[:, head_base * DH : head_base * DH + CSL].astype(bf)
                ),
                "wo": np.ascontiguousarray(W_o[perm, :].astype(bf)),
                "bk": np.ascontiguousarray(b_kq[perm]),
                "bq": np.ascontiguousarray(b_kq[D + perm]),
            }
        )
    return in_maps


def assemble(results, b_v, W_o, b_o):
    bias_row = (b_v.astype(np.float64) @ W_o.astype(np.float64) + b_o).astype(
        np.float32
    )
    out = np.zeros((B, L, D), dtype=np.float32)
    for c in range(NCORES):
        out[c // 4] += np.asarray(results[c]["y"]).astype(np.float32)
    out += bias_row[None, None, :]
    return out


def kernel(x, W_kq, b_kq, W_v, b_v, W_o, b_o):
    x = np.asarray(x, dtype=np.float32)
    W_kq = np.asarray(W_kq, dtype=np.float32)
    b_kq = np.asarray(b_kq, dtype=np.float32)
    W_v = np.asarray(W_v, dtype=np.float32)
    b_v = np.asarray(b_v, dtype=np.float32)
    W_o = np.asarray(W_o, dtype=np.float32)
    b_o = np.asarray(b_o, dtype=np.float32)

    nc = _get_nc()
    in_maps = make_in_maps(x, W_kq, b_kq, W_v, b_v, W_o, b_o)
    res = run_bass_kernel_spmd(nc, in_maps, core_ids=list(range(NCORES)))
    return assemble(res.results, b_v, W_o, b_o)
